# revision 41
# baseline (speedup 1.0000x reference)
"""Multi-head causal attention (seq=4096, d_model=1024, 16 heads) on 8
Trainium2 NeuronCores.

Sharding: tensor-parallel over heads. Core c owns heads 2c, 2c+1 (128 of the
1024 hidden dims). Each core computes its Q/K/V projections (columns of
Wq/Wk/Wv), attention for its two heads, and its slice of the output
projection (rows of Wo); the host sums the 8 partial outputs (the
"all-reduce") and transposes back.

Device layout choices:
 - All matmul inputs are bf16 (f32 PSUM accumulation), halving HBM traffic.
 - Q/K are produced transposed ([d_head, seq]) so score matmuls contract
   over d on the partition axis; the two heads run as concurrent 64-row
   tile_position row-tiles of the PE array.
 - Scores are computed transposed ([k, q]) so softmax'd weights feed the
   attn@V matmul directly as the moving operand, V (augmented with a ones
   column -> softmax denominator for free) as the stationary one; output
   arrives as out^T [d, q], which is exactly what the Wo matmul consumes.
 - exp() has no max-subtraction: scores are ~N(0,1) here and f32 exp is
   safe; softmax is shift-invariant so the result is identical.
 - Causal masking: fully-masked 128x512 score blocks are skipped outright;
   diagonal blocks are multiplied post-exp by 0/1 patterns computed on the
   host from the actual mask input (so any mask works, not just tril).
"""

import sys
import types
from functools import lru_cache

import numpy as np
import ml_dtypes

import concourse.bass as bass
import concourse.mybir as mybir
import concourse.tile as tile
import concourse.bass_utils as bass_utils
from concourse.bass_utils import run_bass_kernel_spmd
from concourse.vector_clock import ScopedClock
from concourse.masks import make_identity

# Let walrus dedupe/pipeline LDWEIGHTS (off by default in this harness).
_orig_bvo = bass_utils.bir_verify_and_optimise


def _bvo_ldwopt(*args, **kwargs):
    import concourse.bass_utils as bu

    orig_run = bu.run_command

    def patched_run(argv, **kw):
        argv = [
            a.replace("--enable-ldw-opt=false", "--enable-ldw-opt=true")
            if isinstance(a, str)
            else a
            for a in argv
        ]
        return orig_run(argv, **kw)

    bu.run_command = patched_run
    try:
        return _orig_bvo(*args, **kwargs)
    finally:
        bu.run_command = orig_run


import os as _os

if _os.environ.get("LDWOPT", "0") == "1":
    bass_utils.bir_verify_and_optimise = _bvo_ldwopt

SEQ = 4096
D = 1024
NCORES = 8
DH = 64          # head dim
DD = 128         # hidden dims per core (2 heads)
QT = 512         # q tile (free dim of score matmuls)
KB = 128         # k block (partition dim of transposed scores)
NQT = SEQ // QT  # 8
NKB = SEQ // KB  # 32
GROUP = 2        # k-blocks per PSUM score group (2 banks x 2 bufs = 4 banks)

bf16 = mybir.dt.bfloat16
f32 = mybir.dt.float32
BF = ml_dtypes.bfloat16

# ---------------------------------------------------------------------------
# Compat shims for running Bass/Tile via axon + neuronxcc in this container.
# ---------------------------------------------------------------------------

_MAXW = 1


def _patched_drain_and_barrier(self, tick_clock, wait_clock):
    # The Tile kernel-tail drain carries one sync-wait per pending semaphore;
    # this neuronxcc build rejects more than a couple of waits per
    # instruction, so spread them over a chain of drains.
    nc = self.nc
    drain_inst = nc.sync.drain()
    wait_clock.add_sem_waits(
        drain_inst.ins, ScopedClock({None: tick_clock.global_clock})
    )
    mi = drain_inst.ins
    waits = list(mi.sync_info.on_wait)
    if len(waits) > _MAXW:
        mi.sync_info = mybir.SyncInfo(
            on_wait=waits[:_MAXW], on_update=list(mi.sync_info.on_update)
        )
        rest = waits[_MAXW:]
        for i in range(0, len(rest), _MAXW):
            extra = nc.sync.drain()
            extra.ins.sync_info = mybir.SyncInfo(
                on_wait=rest[i : i + _MAXW], on_update=[]
            )
    nc.all_engine_barrier()
    popped = nc._tile_sem_poison_stack.pop()
    assert popped is self._sem_poison
    nc.clear_and_free_semaphores(list(self.sems.allocated().values()))
    nc.all_engine_barrier()


tile.TileContext._drain_and_barrier = _patched_drain_and_barrier


def _split_sync_waits(nc, maxw=_MAXW):
    """Move excess semaphore waits onto same-engine EventSemaphore carriers
    inserted just before the over-subscribed instruction. Data sems are
    monotonic within the kernel, so this is semantics-preserving."""
    n = 0
    for func in nc.m.functions:
        for block in func.blocks:
            insts = list(block.instructions)
            out = []
            changed = False
            for inst in insts:
                si = inst.sync_info
                if si is None:
                    out.append(inst)
                    continue
                waits = list(si.on_wait)
                if len(waits) > maxw:
                    extra, keep = waits[:-maxw], waits[-maxw:]
                    for i in range(0, len(extra), maxw):
                        carrier = mybir.InstEventSemaphore(
                            name=f"I-waitsplit-{n}", ins=[], outs=[]
                        )
                        n += 1
                        carrier.engine = inst.engine
                        carrier.sync_info = mybir.SyncInfo(
                            on_wait=extra[i : i + maxw], on_update=[]
                        )
                        out.append(carrier)
                    inst.sync_info = mybir.SyncInfo(
                        on_wait=keep, on_update=list(si.on_update)
                    )
                    changed = True
                out.append(inst)
            if changed:
                block.instructions = out


# ---------------------------------------------------------------------------
# Mask analysis (host side)
# ---------------------------------------------------------------------------


def _analyze_mask(mask2: np.ndarray):
    """Classify each (qt, kb) score block of mask2[q, k] as full / empty /
    partial. Returns (patterns [n,128,512] bf16 in [k, q] layout,
    blocks[qt] = list of (kb, pat_idx or None))."""
    pats = []
    pat_ids = {}
    blocks = []
    m = mask2 != 0
    for qt in range(NQT):
        sub_q = m[qt * QT : (qt + 1) * QT, :]
        col_any = sub_q.any(axis=0)
        col_all = sub_q.all(axis=0)
        row = []
        for kb in range(NKB):
            ca = col_any[kb * KB : (kb + 1) * KB]
            if not ca.any():
                continue
            if col_all[kb * KB : (kb + 1) * KB].all():
                row.append((kb, None))
                continue
            pat = np.ascontiguousarray(
                sub_q[:, kb * KB : (kb + 1) * KB].T
            ).astype(BF)  # [128 k, 512 q]
            key = pat.tobytes()
            if key not in pat_ids:
                pat_ids[key] = len(pats)
                pats.append(pat)
            row.append((kb, pat_ids[key]))
        blocks.append(row)
    if pats:
        patterns = np.stack(pats)
    else:
        patterns = np.zeros((1, KB, QT), BF)  # dummy (unused)
    return patterns, blocks


# ---------------------------------------------------------------------------
# Kernel build
# ---------------------------------------------------------------------------


def _build(blocks_key, n_pat, blocks):
    nc = bass.Bass("TRN2", target_bir_lowering=False, debug=False)

    qT_d = nc.dram_tensor("qT", [D, SEQ], bf16, kind="ExternalInput").ap()
    kT_d = nc.dram_tensor("kT", [D, SEQ], bf16, kind="ExternalInput").ap()
    vT_d = nc.dram_tensor("vT", [D, SEQ], bf16, kind="ExternalInput").ap()
    wq_d = nc.dram_tensor("wq", [D, DD], bf16, kind="ExternalInput").ap()
    wk_d = nc.dram_tensor("wk", [D, DD], bf16, kind="ExternalInput").ap()
    wv_d = nc.dram_tensor("wv", [D, DD], bf16, kind="ExternalInput").ap()
    w2_d = nc.dram_tensor("w2", [DD, D], bf16, kind="ExternalInput").ap()
    pm_d = nc.dram_tensor("pmasks", [n_pat, KB, QT], bf16, kind="ExternalInput").ap()
    out_d = nc.dram_tensor("outT", [D, SEQ], bf16, kind="ExternalOutput").ap()

    NF = D // 128  # 8 contraction chunks for the projections
    resident_masks = n_pat <= 24

    with tile.TileContext(nc) as tc:
        import contextlib

        with contextlib.ExitStack() as stk:
            constp = stk.enter_context(tc.tile_pool(name="const", bufs=1))
            sb = stk.enter_context(tc.tile_pool(name="sb", bufs=1))
            stream = stk.enter_context(tc.tile_pool(name="stream", bufs=4))
            oep = stk.enter_context(tc.tile_pool(name="oep", bufs=3))
            expp = stk.enter_context(tc.tile_pool(name="expp", bufs=4))
            bcp = stk.enter_context(tc.tile_pool(name="bcp", bufs=3))
            rcp = stk.enter_context(tc.tile_pool(name="rcp", bufs=3))
            obp = stk.enter_context(tc.tile_pool(name="obp", bufs=4))
            drp = stk.enter_context(tc.tile_pool(name="drp", bufs=2, space="DRAM"))

            ident = constp.tile([128, 128], bf16)
            make_identity(nc, ident)

            wq_s = constp.tile([128, NF * DD], bf16, tag="wq")
            wk_s = constp.tile([128, NF * DD], bf16, tag="wk")
            wv_s = constp.tile([128, NF * DD], bf16, tag="wv")
            w2_s = constp.tile([DD, D], bf16, tag="w2")
            # Constants ride the (otherwise idle) GpSimd DMA queue so the
            # first projection's chunk loads own the sync queue from t=0.
            for w_s, w_d in ((wk_s, wk_d), (wv_s, wv_d), (wq_s, wq_d)):
                nc.gpsimd.dma_start(
                    out=w_s.rearrange("p (f m) -> p f m", f=NF),
                    in_=w_d.rearrange("(f p) m -> p f m", p=128),
                )
            nc.gpsimd.dma_start(out=w2_s, in_=w2_d)

            if resident_masks:
                pmask_s = constp.tile([128, n_pat * QT], bf16, tag="pm")
                for i in range(n_pat):
                    nc.gpsimd.dma_start(
                        out=pmask_s[:, i * QT : (i + 1) * QT], in_=pm_d[i]
                    )

            q_T = sb.tile([DD, SEQ], bf16, tag="q_T")
            k_T = sb.tile([DD, SEQ], bf16, tag="k_T")
            # Copies with the two 64-partition head halves swapped: score
            # matmuls alternate between the natural and swapped copies so
            # consecutive LDWEIGHTS hit disjoint PE row groups and overlap
            # with the in-flight matmul (measured 427ns -> ~110ns per mm).
            q_Tsw = sb.tile([DD, SEQ], bf16, tag="q_Tsw")
            k_Tsw = sb.tile([DD, SEQ], bf16, tag="k_Tsw")
            v_T = sb.tile([DD, SEQ], bf16, tag="v_T")
            vaug = sb.tile([128, NKB * 130], bf16, tag="vaug")
            ohT = sb.tile([DD, SEQ], bf16, tag="ohT")

            vaug3 = vaug.rearrange("p (t c) -> p t c", c=65)
            nc.gpsimd.memset(vaug3[:, :, 64], 1.0)

            # ---- fused pipeline: per s-chunk, project K/V/Q (s-outer
            # accumulation chains, 1 PSUM bank each) and V transposes, then
            # run attention for q-tile qt=s. The DMA-bound projection stream
            # hides behind the ScalarE-bound attention of earlier tiles.
            # PSUM budget: pj 2 + scores 2x2 + out accumulators 2 = 8 banks.
            with (
                tc.tile_pool(name="pj_ps", bufs=2, space="PSUM") as pj_ps,
                tc.tile_pool(name="sc_ps", bufs=2, space="PSUM") as sc_ps,
                tc.tile_pool(name="o_ps", bufs=1, space="PSUM") as o_ps,
                tc.tile_pool(name="pmt", bufs=2) as pmt,
            ):
                # Normalization of q-tile qt is emitted *after* the first
                # score group of qt+1 so the DVE-strict-FIFO reciprocal
                # (~3.3us) never sits ahead of the mask-multiplies the PE is
                # waiting on.
                pending_norm = []

                def flush_norm():
                    while pending_norm:
                        pending_norm.pop(0)()

                def proj_chunk(x_d, w_s, dest, dest_sw, s):
                    sub = stream.tile([128, NF, QT], bf16, tag="sub", name="sub")
                    for hh in range(2):
                        nc.sync.dma_start(
                            out=sub[:, hh * 4 : (hh + 1) * 4, :],
                            in_=x_d[
                                hh * 512 : (hh + 1) * 512, s * QT : (s + 1) * QT
                            ].rearrange("(f p) q -> p f q", p=128),
                        )
                    ps = pj_ps.tile([DD, QT], f32, tag="pj", name="pp")
                    for f in range(NF):
                        nc.tensor.matmul(
                            ps,
                            w_s[:, f * DD : (f + 1) * DD],
                            sub[:, f, :],
                            start=(f == 0),
                            stop=(f == NF - 1),
                        )
                    sl = slice(s * QT, (s + 1) * QT)
                    if s % 2:
                        nc.vector.tensor_copy(dest[:, sl], ps)
                        if dest_sw is not None:
                            nc.scalar.copy(dest_sw[64:128, sl], ps[0:64, :])
                            nc.scalar.copy(dest_sw[0:64, sl], ps[64:128, :])
                    else:
                        nc.scalar.copy(dest[:, sl], ps)
                        if dest_sw is not None:
                            nc.vector.tensor_copy(dest_sw[64:128, sl], ps[0:64, :])
                            nc.vector.tensor_copy(dest_sw[0:64, sl], ps[64:128, :])

                for s in range(NQT):
                    proj_chunk(kT_d, wk_s, k_T, k_Tsw, s)
                    proj_chunk(vT_d, wv_s, v_T, None, s)
                    for kb in range(4 * s, 4 * s + 4):
                        pt = pj_ps.tile([128, 128], bf16, tag="pj", name="pt")
                        nc.tensor.transpose(
                            pt, v_T[:, kb * 128 : (kb + 1) * 128], ident
                        )
                        nc.vector.tensor_copy(
                            vaug[:, kb * 130 : kb * 130 + 64], pt[:, 0:64]
                        )
                        nc.vector.tensor_copy(
                            vaug[:, kb * 130 + 65 : kb * 130 + 129], pt[:, 64:128]
                        )
                    proj_chunk(qT_d, wq_s, q_T, q_Tsw, s)

                    qt = s
                    kbs = blocks[qt]
                    if not kbs:
                        continue
                    outT = [
                        o_ps.tile([65, QT], f32, tag=f"o{h}", name=f"outT{h}")
                        for h in range(2)
                    ]
                    first_kb = kbs[0][0]
                    last_kb = kbs[-1][0]
                    for g in range(0, len(kbs), GROUP):
                        if g == GROUP:
                            flush_norm()
                        grp = kbs[g : g + GROUP]
                        for h in range(2):
                            sc = sc_ps.tile([128, GROUP * QT], f32, tag="sc")
                            for i, (kb, _pi) in enumerate(grp):
                                # alternate natural/swapped copies by kb parity
                                if (kb + h) % 2 == 0:
                                    kt_src, qt_src, p0 = k_T, q_T, 64 * h
                                else:
                                    kt_src, qt_src, p0 = k_Tsw, q_Tsw, 64 * (1 - h)
                                nc.tensor.matmul(
                                    sc[:, i * QT : (i + 1) * QT],
                                    kt_src[p0 : p0 + 64, kb * KB : (kb + 1) * KB],
                                    qt_src[p0 : p0 + 64, qt * QT : (qt + 1) * QT],
                                    start=True,
                                    stop=True,
                                    tile_position=(p0, 0),
                                )
                            ex = expp.tile([128, GROUP * QT], bf16, tag="exp")
                            nw = len(grp) * QT
                            nc.scalar.activation(
                                ex[:, :nw],
                                sc[:, :nw],
                                mybir.ActivationFunctionType.Exp,
                                scale=0.125,
                            )
                            for i, (kb, pi) in enumerate(grp):
                                if pi is None:
                                    continue
                                if resident_masks:
                                    msk = pmask_s[:, pi * QT : (pi + 1) * QT]
                                else:
                                    mt = pmt.tile([128, QT], bf16, tag="pmt")
                                    nc.sync.dma_start(out=mt, in_=pm_d[pi])
                                    msk = mt
                                nc.vector.tensor_mul(
                                    ex[:, i * QT : (i + 1) * QT],
                                    ex[:, i * QT : (i + 1) * QT],
                                    msk,
                                )
                            for i, (kb, _pi) in enumerate(grp):
                                nc.tensor.matmul(
                                    outT[h],
                                    vaug[:, kb * 130 + 65 * h : kb * 130 + 65 * h + 65],
                                    ex[:, i * QT : (i + 1) * QT],
                                    start=(kb == first_kb),
                                    stop=(kb == last_kb),
                                )
                    for h in range(2):
                        # Evacuate the PSUM accumulator right away (frees the
                        # bank for the next q tile); normalization is deferred
                        # into the next q tile's instruction stream.
                        oe = oep.tile([65, QT], f32, tag=f"oe{h}")
                        nc.vector.tensor_copy(oe, outT[h])

                        def norm(h=h, qt=qt, oe=oe):
                            recip = rcp.tile([1, QT], f32, tag=f"rc{h}", name="recip")
                            nc.vector.reciprocal(recip, oe[64:65, :])
                            rd = drp.tile([1, QT], f32, tag="rd", name="rd")
                            nc.sync.dma_start(out=rd, in_=recip)
                            bc = bcp.tile([64, QT], f32, tag="bc", name="bc")
                            bc_src = bass.AP(
                                tensor=rd.tensor,
                                offset=rd.offset,
                                ap=[[0, 64]] + list(rd.ap[1:]),
                            )
                            nc.sync.dma_start(out=bc, in_=bc_src)
                            nc.vector.tensor_mul(
                                ohT[64 * h : 64 * h + 64, qt * QT : (qt + 1) * QT],
                                oe[0:64, :],
                                bc,
                            )

                        pending_norm.append(norm)
                flush_norm()

            # ---- output projection: partial^T[m, s] = W2^T @ ohT ----
            with tc.tile_pool(name="wo_ps", bufs=4, space="PSUM") as wo_ps:
                for mt in range(D // 128):
                    ob = obp.tile([128, SEQ], bf16, tag="ob")
                    for st in range(NQT):
                        wp = wo_ps.tile([128, QT], f32, tag="wo", name="wp")
                        nc.tensor.matmul(
                            wp,
                            w2_s[:, mt * 128 : (mt + 1) * 128],
                            ohT[:, st * QT : (st + 1) * QT],
                            start=True,
                            stop=True,
                        )
                        if st % 2:
                            nc.vector.tensor_copy(
                                ob[:, st * QT : (st + 1) * QT], wp
                            )
                        else:
                            nc.scalar.copy(ob[:, st * QT : (st + 1) * QT], wp)
                    nc.sync.dma_start(
                        out=out_d[mt * 128 : (mt + 1) * 128, :], in_=ob
                    )

    return nc


_NC_CACHE = {}


def _get_nc(mask2, split=True):
    key = hash(mask2.tobytes())
    if key not in _NC_CACHE:
        patterns, blocks = _analyze_mask(mask2)
        nc = _build(key, patterns.shape[0], blocks)
        _NC_CACHE[key] = [nc, patterns, False]
    ent = _NC_CACHE[key]
    if split and not ent[2]:
        _split_sync_waits(ent[0])
        ent[2] = True
    return ent[0], ent[1]


# ---------------------------------------------------------------------------
# Entry point
# ---------------------------------------------------------------------------


def kernel(q, k, v, mask, Wq, Wk, Wv, Wo):
    q = np.asarray(q, np.float32)
    k = np.asarray(k, np.float32)
    v = np.asarray(v, np.float32)
    mask2 = np.asarray(mask)[0, 0]
    Wq = np.asarray(Wq, np.float32)
    Wk = np.asarray(Wk, np.float32)
    Wv = np.asarray(Wv, np.float32)
    Wo = np.asarray(Wo, np.float32)

    nc, patterns = _get_nc(mask2)

    qT = np.ascontiguousarray(q[0].T).astype(BF)
    kT = np.ascontiguousarray(k[0].T).astype(BF)
    vT = np.ascontiguousarray(v[0].T).astype(BF)

    in_maps = []
    for c in range(NCORES):
        sl = slice(c * DD, (c + 1) * DD)
        in_maps.append(
            {
                "qT": qT,
                "kT": kT,
                "vT": vT,
                "wq": np.ascontiguousarray(Wq[sl, :].T).astype(BF),
                "wk": np.ascontiguousarray(Wk[sl, :].T).astype(BF),
                "wv": np.ascontiguousarray(Wv[sl, :].T).astype(BF),
                "w2": np.ascontiguousarray(Wo[:, sl].T).astype(BF),
                "pmasks": patterns,
            }
        )

    res = run_bass_kernel_spmd(nc, in_maps, core_ids=list(range(NCORES)))
    acc = np.zeros((D, SEQ), np.float32)
    for r in res.results:
        acc += np.asarray(r["outT"], dtype=np.float32)
    return np.ascontiguousarray(acc.T)[None, :, :]


# revision 42
# speedup vs baseline: 1.1021x; 1.1021x over previous
"""Multi-head causal attention (seq=4096, d_model=1024, 16 heads) on 8
Trainium2 NeuronCores.

Sharding: tensor-parallel over heads. Core c owns heads 2c, 2c+1 (128 of the
1024 hidden dims). Each core computes its Q/K/V projections (columns of
Wq/Wk/Wv), attention for its two heads, and its slice of the output
projection (rows of Wo); the host sums the 8 partial outputs (the
"all-reduce") and transposes back.

Device layout choices:
 - All matmul inputs are bf16 (f32 PSUM accumulation), halving HBM traffic.
 - Q/K are produced transposed ([d_head, seq]) so score matmuls contract
   over d on the partition axis; the two heads run as concurrent 64-row
   tile_position row-tiles of the PE array.
 - Scores are computed transposed ([k, q]) so softmax'd weights feed the
   attn@V matmul directly as the moving operand, V (augmented with a ones
   column -> softmax denominator for free) as the stationary one; output
   arrives as out^T [d, q], which is exactly what the Wo matmul consumes.
 - exp() has no max-subtraction: scores are ~N(0,1) here and f32 exp is
   safe; softmax is shift-invariant so the result is identical.
 - Causal masking: fully-masked 128x512 score blocks are skipped outright;
   diagonal blocks are multiplied post-exp by 0/1 patterns computed on the
   host from the actual mask input (so any mask works, not just tril).
"""

import sys
import types
from functools import lru_cache

import numpy as np
import ml_dtypes

import concourse.bass as bass
import concourse.mybir as mybir
import concourse.tile as tile
import concourse.bass_utils as bass_utils
from concourse.bass_utils import run_bass_kernel_spmd
from concourse.vector_clock import ScopedClock
from concourse.masks import make_identity

# Let walrus dedupe/pipeline LDWEIGHTS (off by default in this harness).
_orig_bvo = bass_utils.bir_verify_and_optimise


def _bvo_ldwopt(*args, **kwargs):
    import concourse.bass_utils as bu

    orig_run = bu.run_command

    def patched_run(argv, **kw):
        argv = [
            a.replace("--enable-ldw-opt=false", "--enable-ldw-opt=true")
            if isinstance(a, str)
            else a
            for a in argv
        ]
        return orig_run(argv, **kw)

    bu.run_command = patched_run
    try:
        return _orig_bvo(*args, **kwargs)
    finally:
        bu.run_command = orig_run


import os as _os

if _os.environ.get("LDWOPT", "0") == "1":
    bass_utils.bir_verify_and_optimise = _bvo_ldwopt

SEQ = 4096
D = 1024
NCORES = 8
DH = 64          # head dim
DD = 128         # hidden dims per core (2 heads)
QT = 512         # q tile (free dim of score matmuls)
KB = 128         # k block (partition dim of transposed scores)
NQT = SEQ // QT  # 8
NKB = SEQ // KB  # 32
GROUP = 3        # k-blocks per PSUM score group (3 banks; x2 heads = 6 banks)

bf16 = mybir.dt.bfloat16
f32 = mybir.dt.float32
BF = ml_dtypes.bfloat16

# ---------------------------------------------------------------------------
# Compat shims for running Bass/Tile via axon + neuronxcc in this container.
# ---------------------------------------------------------------------------

_MAXW = 1


def _patched_drain_and_barrier(self, tick_clock, wait_clock):
    # The Tile kernel-tail drain carries one sync-wait per pending semaphore;
    # this neuronxcc build rejects more than a couple of waits per
    # instruction, so spread them over a chain of drains.
    nc = self.nc
    drain_inst = nc.sync.drain()
    wait_clock.add_sem_waits(
        drain_inst.ins, ScopedClock({None: tick_clock.global_clock})
    )
    mi = drain_inst.ins
    waits = list(mi.sync_info.on_wait)
    if len(waits) > _MAXW:
        mi.sync_info = mybir.SyncInfo(
            on_wait=waits[:_MAXW], on_update=list(mi.sync_info.on_update)
        )
        rest = waits[_MAXW:]
        for i in range(0, len(rest), _MAXW):
            extra = nc.sync.drain()
            extra.ins.sync_info = mybir.SyncInfo(
                on_wait=rest[i : i + _MAXW], on_update=[]
            )
    nc.all_engine_barrier()
    popped = nc._tile_sem_poison_stack.pop()
    assert popped is self._sem_poison
    nc.clear_and_free_semaphores(list(self.sems.allocated().values()))
    nc.all_engine_barrier()


tile.TileContext._drain_and_barrier = _patched_drain_and_barrier


def _split_sync_waits(nc, maxw=_MAXW):
    """Move excess semaphore waits onto same-engine EventSemaphore carriers
    inserted just before the over-subscribed instruction. Data sems are
    monotonic within the kernel, so this is semantics-preserving."""
    n = 0
    for func in nc.m.functions:
        for block in func.blocks:
            insts = list(block.instructions)
            out = []
            changed = False
            for inst in insts:
                si = inst.sync_info
                if si is None:
                    out.append(inst)
                    continue
                waits = list(si.on_wait)
                if len(waits) > maxw:
                    extra, keep = waits[:-maxw], waits[-maxw:]
                    for i in range(0, len(extra), maxw):
                        carrier = mybir.InstEventSemaphore(
                            name=f"I-waitsplit-{n}", ins=[], outs=[]
                        )
                        n += 1
                        carrier.engine = inst.engine
                        carrier.sync_info = mybir.SyncInfo(
                            on_wait=extra[i : i + maxw], on_update=[]
                        )
                        out.append(carrier)
                    inst.sync_info = mybir.SyncInfo(
                        on_wait=keep, on_update=list(si.on_update)
                    )
                    changed = True
                out.append(inst)
            if changed:
                block.instructions = out


# ---------------------------------------------------------------------------
# Mask analysis (host side)
# ---------------------------------------------------------------------------


def _analyze_mask(mask2: np.ndarray):
    """Classify each (qt, kb) score block of mask2[q, k] as full / empty /
    partial. Returns (patterns [n,128,512] bf16 in [k, q] layout,
    blocks[qt] = list of (kb, pat_idx or None))."""
    pats = []
    pat_ids = {}
    blocks = []
    m = mask2 != 0
    for qt in range(NQT):
        sub_q = m[qt * QT : (qt + 1) * QT, :]
        col_any = sub_q.any(axis=0)
        col_all = sub_q.all(axis=0)
        row = []
        for kb in range(NKB):
            ca = col_any[kb * KB : (kb + 1) * KB]
            if not ca.any():
                continue
            if col_all[kb * KB : (kb + 1) * KB].all():
                row.append((kb, None))
                continue
            pat = np.ascontiguousarray(
                sub_q[:, kb * KB : (kb + 1) * KB].T
            ).astype(BF)  # [128 k, 512 q]
            key = pat.tobytes()
            if key not in pat_ids:
                pat_ids[key] = len(pats)
                pats.append(pat)
            row.append((kb, pat_ids[key]))
        blocks.append(row)
    if pats:
        patterns = np.stack(pats)
    else:
        patterns = np.zeros((1, KB, QT), BF)  # dummy (unused)
    return patterns, blocks


# ---------------------------------------------------------------------------
# Kernel build
# ---------------------------------------------------------------------------


def _build(blocks_key, n_pat, blocks):
    nc = bass.Bass("TRN2", target_bir_lowering=False, debug=False)

    qT_d = nc.dram_tensor("qT", [D, SEQ], bf16, kind="ExternalInput").ap()
    kT_d = nc.dram_tensor("kT", [D, SEQ], bf16, kind="ExternalInput").ap()
    vT_d = nc.dram_tensor("vT", [D, SEQ], bf16, kind="ExternalInput").ap()
    wq_d = nc.dram_tensor("wq", [D, DD], bf16, kind="ExternalInput").ap()
    wk_d = nc.dram_tensor("wk", [D, DD], bf16, kind="ExternalInput").ap()
    wv_d = nc.dram_tensor("wv", [D, DD], bf16, kind="ExternalInput").ap()
    w2_d = nc.dram_tensor("w2", [DD, D], bf16, kind="ExternalInput").ap()
    pm_d = nc.dram_tensor("pmasks", [n_pat, KB, QT], bf16, kind="ExternalInput").ap()
    out_d = nc.dram_tensor("outT", [D, SEQ], bf16, kind="ExternalOutput").ap()

    NF = D // 128  # 8 contraction chunks for the projections
    resident_masks = n_pat <= 24

    with tile.TileContext(nc) as tc:
        import contextlib

        with contextlib.ExitStack() as stk:
            constp = stk.enter_context(tc.tile_pool(name="const", bufs=1))
            sb = stk.enter_context(tc.tile_pool(name="sb", bufs=1))
            stream = stk.enter_context(tc.tile_pool(name="stream", bufs=4))
            oep = stk.enter_context(tc.tile_pool(name="oep", bufs=3))
            expp = stk.enter_context(tc.tile_pool(name="expp", bufs=4))
            bcp = stk.enter_context(tc.tile_pool(name="bcp", bufs=3))
            rcp = stk.enter_context(tc.tile_pool(name="rcp", bufs=3))
            obp = stk.enter_context(tc.tile_pool(name="obp", bufs=4))
            drp = stk.enter_context(tc.tile_pool(name="drp", bufs=2, space="DRAM"))

            ident = constp.tile([128, 128], bf16)
            make_identity(nc, ident)

            wq_s = constp.tile([128, NF * DD], bf16, tag="wq")
            wk_s = constp.tile([128, NF * DD], bf16, tag="wk")
            wv_s = constp.tile([128, NF * DD], bf16, tag="wv")
            w2_s = constp.tile([DD, D], bf16, tag="w2")
            # Constants ride the (otherwise idle) GpSimd DMA queue so the
            # first projection's chunk loads own the sync queue from t=0.
            for w_s, w_d in ((wk_s, wk_d), (wv_s, wv_d), (wq_s, wq_d)):
                nc.gpsimd.dma_start(
                    out=w_s.rearrange("p (f m) -> p f m", f=NF),
                    in_=w_d.rearrange("(f p) m -> p f m", p=128),
                )
            nc.gpsimd.dma_start(out=w2_s, in_=w2_d)

            if resident_masks:
                pmask_s = constp.tile([128, n_pat * QT], bf16, tag="pm")
                for i in range(n_pat):
                    nc.gpsimd.dma_start(
                        out=pmask_s[:, i * QT : (i + 1) * QT], in_=pm_d[i]
                    )

            q_T = sb.tile([DD, SEQ], bf16, tag="q_T")
            k_T = sb.tile([DD, SEQ], bf16, tag="k_T")
            # Copies with the two 64-partition head halves swapped: score
            # matmuls alternate between the natural and swapped copies so
            # consecutive LDWEIGHTS hit disjoint PE row groups and overlap
            # with the in-flight matmul (measured 427ns -> ~110ns per mm).
            q_Tsw = sb.tile([DD, SEQ], bf16, tag="q_Tsw")
            k_Tsw = sb.tile([DD, SEQ], bf16, tag="k_Tsw")
            v_T = sb.tile([DD, SEQ], bf16, tag="v_T")
            vaug = sb.tile([128, NKB * 130], bf16, tag="vaug")
            ohT = sb.tile([DD, SEQ], bf16, tag="ohT")

            vaug3 = vaug.rearrange("p (t c) -> p t c", c=65)
            nc.gpsimd.memset(vaug3[:, :, 64], 1.0)

            # ---- projections: dest_T[dd, s] = sum_f W[dd, f] * x[s, f] ----
            with tc.tile_pool(name="proj_ps", bufs=8, space="PSUM") as proj_ps:
                for x_d, w_s, dest, dest_sw in (
                    (kT_d, wk_s, k_T, k_Tsw),
                    (vT_d, wv_s, v_T, None),
                    (qT_d, wq_s, q_T, q_Tsw),
                ):
                    psums = [
                        proj_ps.tile([DD, QT], f32, tag="proj", name=f"proj{s}")
                        for s in range(NQT)
                    ]
                    for f in range(NF):
                        ch = stream.tile([128, SEQ], bf16, tag="chunk")
                        qtr = SEQ // 4
                        for qd in range(4):
                            nc.sync.dma_start(
                                out=ch[:, qd * qtr : (qd + 1) * qtr],
                                in_=x_d[f * 128 : (f + 1) * 128, qd * qtr : (qd + 1) * qtr],
                            )
                        for s in range(NQT):
                            nc.tensor.matmul(
                                psums[s],
                                w_s[:, f * DD : (f + 1) * DD],
                                ch[:, s * QT : (s + 1) * QT],
                                start=(f == 0),
                                stop=(f == NF - 1),
                            )
                    for s in range(NQT):
                        if s % 2:
                            nc.vector.tensor_copy(
                                dest[:, s * QT : (s + 1) * QT], psums[s]
                            )
                        else:
                            nc.scalar.copy(dest[:, s * QT : (s + 1) * QT], psums[s])
                        if dest_sw is not None:
                            sl = slice(s * QT, (s + 1) * QT)
                            if s % 2:
                                nc.scalar.copy(dest_sw[64:128, sl], psums[s][0:64, :])
                                nc.scalar.copy(dest_sw[0:64, sl], psums[s][64:128, :])
                            else:
                                nc.vector.tensor_copy(
                                    dest_sw[64:128, sl], psums[s][0:64, :]
                                )
                                nc.vector.tensor_copy(
                                    dest_sw[0:64, sl], psums[s][64:128, :]
                                )

            # ---- V_T -> V_aug via PE transposes ----
            with tc.tile_pool(name="tr_ps", bufs=2, space="PSUM") as tr_ps:
                for kb in range(NKB):
                    pt = tr_ps.tile([128, 128], bf16, tag="tr")
                    nc.tensor.transpose(
                        pt, v_T[:, kb * 128 : (kb + 1) * 128], ident
                    )
                    nc.vector.tensor_copy(
                        vaug[:, kb * 130 : kb * 130 + 64], pt[:, 0:64]
                    )
                    nc.vector.tensor_copy(
                        vaug[:, kb * 130 + 65 : kb * 130 + 129], pt[:, 64:128]
                    )

            # ---- attention ----
            with (
                tc.tile_pool(name="sc_ps", bufs=2, space="PSUM") as sc_ps,
                tc.tile_pool(name="o_ps", bufs=1, space="PSUM") as o_ps,
                tc.tile_pool(name="pmt", bufs=2) as pmt,
            ):
                # Normalization of q-tile qt is emitted *after* the first
                # score group of qt+1 so the DVE-strict-FIFO reciprocal
                # (~3.3us) never sits ahead of the mask-multiplies the PE is
                # waiting on.
                pending_norm = []

                def flush_norm():
                    while pending_norm:
                        pending_norm.pop(0)()

                for qt in range(NQT):
                    kbs = blocks[qt]
                    if not kbs:
                        continue
                    outT = [
                        o_ps.tile([65, QT], f32, tag=f"o{h}", name=f"outT{h}")
                        for h in range(2)
                    ]
                    first_kb = kbs[0][0]
                    last_kb = kbs[-1][0]
                    for g in range(0, len(kbs), GROUP):
                        if g == GROUP:
                            flush_norm()
                        grp = kbs[g : g + GROUP]
                        for h in range(2):
                            sc = sc_ps.tile([128, GROUP * QT], f32, tag="sc")
                            for i, (kb, _pi) in enumerate(grp):
                                # alternate natural/swapped copies by kb parity
                                if (kb + h) % 2 == 0:
                                    kt_src, qt_src, p0 = k_T, q_T, 64 * h
                                else:
                                    kt_src, qt_src, p0 = k_Tsw, q_Tsw, 64 * (1 - h)
                                nc.tensor.matmul(
                                    sc[:, i * QT : (i + 1) * QT],
                                    kt_src[p0 : p0 + 64, kb * KB : (kb + 1) * KB],
                                    qt_src[p0 : p0 + 64, qt * QT : (qt + 1) * QT],
                                    start=True,
                                    stop=True,
                                    tile_position=(p0, 0),
                                )
                            ex = expp.tile([128, GROUP * QT], bf16, tag="exp")
                            nw = len(grp) * QT
                            nc.scalar.activation(
                                ex[:, :nw],
                                sc[:, :nw],
                                mybir.ActivationFunctionType.Exp,
                                scale=0.125,
                            )
                            for i, (kb, pi) in enumerate(grp):
                                if pi is None:
                                    continue
                                if resident_masks:
                                    msk = pmask_s[:, pi * QT : (pi + 1) * QT]
                                else:
                                    mt = pmt.tile([128, QT], bf16, tag="pmt")
                                    nc.sync.dma_start(out=mt, in_=pm_d[pi])
                                    msk = mt
                                nc.vector.tensor_mul(
                                    ex[:, i * QT : (i + 1) * QT],
                                    ex[:, i * QT : (i + 1) * QT],
                                    msk,
                                )
                            for i, (kb, _pi) in enumerate(grp):
                                nc.tensor.matmul(
                                    outT[h],
                                    vaug[:, kb * 130 + 65 * h : kb * 130 + 65 * h + 65],
                                    ex[:, i * QT : (i + 1) * QT],
                                    start=(kb == first_kb),
                                    stop=(kb == last_kb),
                                )
                    for h in range(2):
                        # Evacuate the PSUM accumulator right away (frees the
                        # bank for the next q tile); normalization is deferred
                        # into the next q tile's instruction stream.
                        oe = oep.tile([65, QT], f32, tag=f"oe{h}")
                        nc.vector.tensor_copy(oe, outT[h])

                        def norm(h=h, qt=qt, oe=oe):
                            recip = rcp.tile([1, QT], f32, tag=f"rc{h}", name="recip")
                            nc.vector.reciprocal(recip, oe[64:65, :])
                            rd = drp.tile([1, QT], f32, tag="rd", name="rd")
                            nc.sync.dma_start(out=rd, in_=recip)
                            bc = bcp.tile([64, QT], f32, tag="bc", name="bc")
                            bc_src = bass.AP(
                                tensor=rd.tensor,
                                offset=rd.offset,
                                ap=[[0, 64]] + list(rd.ap[1:]),
                            )
                            nc.sync.dma_start(out=bc, in_=bc_src)
                            nc.vector.tensor_mul(
                                ohT[64 * h : 64 * h + 64, qt * QT : (qt + 1) * QT],
                                oe[0:64, :],
                                bc,
                            )

                        pending_norm.append(norm)
                flush_norm()

            # ---- output projection: partial^T[m, s] = W2^T @ ohT ----
            with tc.tile_pool(name="wo_ps", bufs=4, space="PSUM") as wo_ps:
                for mt in range(D // 128):
                    ob = obp.tile([128, SEQ], bf16, tag="ob")
                    for st in range(NQT):
                        wp = wo_ps.tile([128, QT], f32, tag="wo", name="wp")
                        nc.tensor.matmul(
                            wp,
                            w2_s[:, mt * 128 : (mt + 1) * 128],
                            ohT[:, st * QT : (st + 1) * QT],
                            start=True,
                            stop=True,
                        )
                        if st % 2:
                            nc.vector.tensor_copy(
                                ob[:, st * QT : (st + 1) * QT], wp
                            )
                        else:
                            nc.scalar.copy(ob[:, st * QT : (st + 1) * QT], wp)
                    nc.sync.dma_start(
                        out=out_d[mt * 128 : (mt + 1) * 128, :], in_=ob
                    )

    return nc


_NC_CACHE = {}


def _get_nc(mask2, split=True):
    key = hash(mask2.tobytes())
    if key not in _NC_CACHE:
        patterns, blocks = _analyze_mask(mask2)
        nc = _build(key, patterns.shape[0], blocks)
        _NC_CACHE[key] = [nc, patterns, False]
    ent = _NC_CACHE[key]
    if split and not ent[2]:
        _split_sync_waits(ent[0])
        ent[2] = True
    return ent[0], ent[1]


# ---------------------------------------------------------------------------
# Entry point
# ---------------------------------------------------------------------------


def kernel(q, k, v, mask, Wq, Wk, Wv, Wo):
    q = np.asarray(q, np.float32)
    k = np.asarray(k, np.float32)
    v = np.asarray(v, np.float32)
    mask2 = np.asarray(mask)[0, 0]
    Wq = np.asarray(Wq, np.float32)
    Wk = np.asarray(Wk, np.float32)
    Wv = np.asarray(Wv, np.float32)
    Wo = np.asarray(Wo, np.float32)

    nc, patterns = _get_nc(mask2)

    qT = np.ascontiguousarray(q[0].T).astype(BF)
    kT = np.ascontiguousarray(k[0].T).astype(BF)
    vT = np.ascontiguousarray(v[0].T).astype(BF)

    in_maps = []
    for c in range(NCORES):
        sl = slice(c * DD, (c + 1) * DD)
        in_maps.append(
            {
                "qT": qT,
                "kT": kT,
                "vT": vT,
                "wq": np.ascontiguousarray(Wq[sl, :].T).astype(BF),
                "wk": np.ascontiguousarray(Wk[sl, :].T).astype(BF),
                "wv": np.ascontiguousarray(Wv[sl, :].T).astype(BF),
                "w2": np.ascontiguousarray(Wo[:, sl].T).astype(BF),
                "pmasks": patterns,
            }
        )

    res = run_bass_kernel_spmd(nc, in_maps, core_ids=list(range(NCORES)))
    acc = np.zeros((D, SEQ), np.float32)
    for r in res.results:
        acc += np.asarray(r["outT"], dtype=np.float32)
    return np.ascontiguousarray(acc.T)[None, :, :]
